# revision 44
# baseline (speedup 1.0000x reference)
"""InterpretableMultiHeadAttention on 8 Trainium2 NeuronCores (Bass/Tile).

Sharding: core c -> batch b = c//2, head-group hg = c%2 (8 of 16 heads).
Math folding (exact up to fp rounding):
  v' = v @ Wv.T + bv, x = sum_h attn_h @ v'_h, out = x @ Wo.T + bo
  Since softmax rows sum to 1:  attn @ (1 bv^T) = 1 bv^T, so
  out = (sum_h attn_h @ v_h) @ (Wo @ Wv).T + (H * Wo @ bv + bo)
The 1/sqrt(d) score scale folds into Wq/bq.  The K-projection bias is
dropped entirely: q.bk is constant along the key axis, so softmax is
invariant to it.

Engine plan: the 33.5M score elements per core must be evacuated
PSUM->SBUF through ScalarE or VectorE; exp rides that move for free.
Chunks alternate between ScalarE (exp activation, fp8e4m3 out) and
VectorE (Schraudolph exp: one tensor_scalar f32->int8 whose bits are the
fp8 approximation).  fp8 e + fp8 v enable DoubleRow PV matmuls (two key
chunks contracted per pass); projections run DoubleRow over fp8 inputs.
Denominators ride the PV matmul as per-head ones-columns, are gathered
from the f16 staging tile via DMA bounce, inverted with the fast approx
reciprocal, and broadcast back.  Finalize (divide + head-sum) runs in
f16 at the DVE's 2x mode; deferred finalize/outproj work is drip-fed
between chunks so the in-order engine queues never burst-stall.
"""
import numpy as np

N_OUT = 1024
N_HEADS = 16
D_K = 64
B = 4
S = 2048
FC = 8          # 1024 contraction f-chunks of 128 (projections)
FP = FC // 2    # DoubleRow f-chunk pairs
PAIRS = 4       # 8 local heads as 4 row-packed pairs
NMM = 512       # matmul moving free dim
JC = S // 128   # key chunks of 128
JP = JC // 2    # DoubleRow key-chunk pairs
IQ = S // NMM   # query blocks of 512
MV = 80         # PV lhsT width (16-aligned): 64 v dims + ones col at 64+h

# Schraudolph exp in fp8e4m3: bitcast_f8(int8(x*8/ln2 + 7*8 + C))
SCH_A = float(8.0 / np.log(2.0))
SCH_B = float(7.0 * 8.0 - 0.24)
DVE_SHARE = 0.45   # fraction of free e-chunks evacuated by VectorE
PV_LAG = 3         # software-pipeline depth: PV trails scores by this many chunk-pairs
WQ_GRACE = 3       # chunk-pairs at block start before deferred work is consumed

_CACHE = {}


def _build_nc():
    from contextlib import ExitStack
    import concourse.bass as bass
    import concourse.bacc as bacc
    import concourse.tile as tile
    import concourse.mybir as mybir

    f16 = mybir.dt.float16
    f32 = mybir.dt.float32
    f8 = mybir.dt.float8e4
    i8 = mybir.dt.int8

    nc = bacc.Bacc("TRN2", target_bir_lowering=False, debug=False, num_devices=8)

    xq_d = nc.dram_tensor("xq", [128, FC, S], f8, kind="ExternalInput")
    xk_d = nc.dram_tensor("xk", [128, FC, S], f8, kind="ExternalInput")
    wq_d = nc.dram_tensor("wq", [128, FC, 512], f8, kind="ExternalInput")
    wk_d = nc.dram_tensor("wk", [128, FC, 512], f8, kind="ExternalInput")
    bq_d = nc.dram_tensor("bq", [128, PAIRS], f32, kind="ExternalInput")
    vv_d = nc.dram_tensor("vv", [PAIRS, 128, JP, 2, 2, MV], f8, kind="ExternalInput")
    wov_d = nc.dram_tensor("wov", [64, N_OUT], f16, kind="ExternalInput")
    out_d = nc.dram_tensor("outT", [8, 128, S], f16, kind="ExternalOutput")
    dg_d = nc.dram_tensor("den_gather", [IQ, 8, NMM], f16)   # raw den bounce
    den_d = nc.dram_tensor("den_scratch", [IQ, 8, NMM], f16)  # recip bounce

    with tile.TileContext(nc) as tc, ExitStack() as ctx:
        const = ctx.enter_context(tc.tile_pool(name="const", bufs=1))
        qkall = ctx.enter_context(tc.tile_pool(name="qkall", bufs=1))
        epool = ctx.enter_context(tc.tile_pool(name="epool", bufs=5))
        blk = ctx.enter_context(tc.tile_pool(name="blk", bufs=2))
        fin = ctx.enter_context(tc.tile_pool(name="fin", bufs=2))
        dpool = ctx.enter_context(tc.tile_pool(name="dpool", bufs=2))
        ost_p = ctx.enter_context(tc.tile_pool(name="ost_p", bufs=2))
        ps_mm = ctx.enter_context(tc.tile_pool(name="ps_mm", bufs=3, space="PSUM"))
        ps_y = ctx.enter_context(tc.tile_pool(name="ps_y", bufs=2, space="PSUM"))
        xctx = ExitStack()
        xstage = xctx.enter_context(tc.tile_pool(name="xstage", bufs=1))

        # ---- input loads: explicit queue plan so proj prerequisites land
        # in consumption order (wq, xq, wk, xk, then vv/wov/bq) ----
        wq_sb = const.tile([128, FC, 512], f8, tag="wq")
        wk_sb = const.tile([128, FC, 512], f8, tag="wk")
        bq_sb = const.tile([128, PAIRS], f32, tag="bq")
        xq_sb = xstage.tile([128, FC, S], f8, tag="xq")
        xk_sb = xstage.tile([128, FC, S], f8, tag="xk")
        nc.sync.dma_start(out=wq_sb[:], in_=wq_d[:])
        qplan = [nc.scalar, nc.gpsimd, nc.scalar, nc.gpsimd]
        for i, f in enumerate(range(0, FC, 2)):
            qplan[i].dma_start(out=xq_sb[:, f:f + 2], in_=xq_d[:, f:f + 2])
        nc.sync.dma_start(out=bq_sb[:], in_=bq_d[:])
        nc.sync.dma_start(out=wk_sb[:], in_=wk_d[:])
        kplan = [nc.gpsimd, nc.scalar, nc.gpsimd, nc.scalar]
        for i, f in enumerate(range(0, FC, 2)):
            kplan[i].dma_start(out=xk_sb[:, f:f + 2], in_=xk_d[:, f:f + 2])
        vv_sb = []
        vq = [nc.sync, nc.scalar, nc.gpsimd, nc.sync]
        for p in range(PAIRS):
            t = qkall.tile([128, JP, 2, 2, MV], f8, tag=f"vv{p}")
            vq[p].dma_start(out=t[:], in_=vv_d[p])
            vv_sb.append(t)
        wov_sb = const.tile([64, N_OUT], f16, tag="wov")
        nc.gpsimd.dma_start(out=wov_sb[:], in_=wov_d[:])

        qT, kT = {}, {}

        def proj(p, is_q):
            if is_q:
                qT[p] = qkall.tile([128, S], f16, tag=f"qT{p}", name=f"qT{p}")
                parts = ((qT[p], wq_sb, xq_sb, True),)
            else:
                kT[p] = qkall.tile([128, S], f16, tag=f"kT{p}", name=f"kT{p}")
                parts = ((kT[p], wk_sb, xk_sb, False),)
            for dst, w_sb, x_sb, is_q in parts:
                for sc in range(S // 1024):
                    ps = ps_mm.tile([128, 1024], f32, tag="mm")
                    for hf in range(2):
                        c0 = sc * 1024 + hf * 512
                        for fp in range(FP):
                            nc.tensor.matmul(
                                out=ps[:, hf * 512:(hf + 1) * 512],
                                lhsT=w_sb[:, 2 * fp:2 * fp + 2,
                                          p * 128:(p + 1) * 128],
                                rhs=x_sb[:, 2 * fp:2 * fp + 2, c0:c0 + 512],
                                start=(fp == 0),
                                stop=(fp == FP - 1),
                                perf_mode=mybir.MatmulPerfMode.DoubleRow,
                            )
                    dsl = dst[:, sc * 1024:(sc + 1) * 1024]
                    if is_q:
                        # bias add rides the PSUM->SBUF evac (VectorE)
                        nc.vector.tensor_scalar_add(
                            out=dsl, in0=ps[:], scalar1=bq_sb[:, p:p + 1])
                    else:
                        # no K bias (softmax-invariant): plain copy on ScalarE
                        nc.scalar.activation(
                            out=dsl, in_=ps[:],
                            func=mybir.ActivationFunctionType.Copy)

        def outproj_ops(iq, y16):
            """One closure per wov slice: MM + evac-cast + store DMA."""
            i0 = iq * NMM

            def one(m):
                po = ps_mm.tile([128, 1024], f32, tag="mm")
                nc.tensor.matmul(
                    out=po[:, :NMM],
                    lhsT=wov_sb[:, m * 128:(m + 1) * 128],
                    rhs=y16[:],
                    start=True, stop=True,
                )
                ost = ost_p.tile([128, NMM], f16, tag="ost")
                if m % 2 == 0:
                    nc.scalar.activation(
                        out=ost[:], in_=po[:, :NMM],
                        func=mybir.ActivationFunctionType.Copy)
                else:
                    nc.vector.tensor_copy(out=ost[:], in_=po[:, :NMM])
                nc.sync.dma_start(out=out_d[m][:, i0:i0 + NMM], in_=ost[:])

            return [lambda m=m: one(m) for m in range(8)]

        def den_chain_ops(iq, h0, h1, rbs_out):
            """Closures: invert gathered denominators for heads [h0,h1) and
            broadcast each head's reciprocal row across 64 partitions.  The
            collect DMA is emitted immediately (latency hides behind the
            chunk stream); the DVE ops drip through work_q so they never
            head-block the evac pipeline."""
            n = h1 - h0
            denr = dpool.tile([n, NMM], f16, tag=f"denr{h0}")
            nc.gpsimd.dma_start(out=denr[:], in_=dg_d[iq, h0:h1])
            denf = dpool.tile([n, NMM], f32, tag=f"denf{h0}")
            denf2 = dpool.tile([n, NMM], f32, tag=f"denf2{h0}")
            den16 = dpool.tile([n, NMM], f16, tag=f"den16{h0}")
            rbs = [fin.tile([64, NMM], f16, tag=f"rb{h}", name=f"rb{h}")
                   for h in range(h0, h1)]
            rbs_out.extend(rbs)

            def bcasts():
                nc.gpsimd.dma_start(out=den_d[iq, h0:h1], in_=den16[:])
                for i, h in enumerate(range(h0, h1)):
                    row = den_d[iq, h:h + 1, :]
                    bc = bass.AP(tensor=row.tensor, offset=row.offset,
                                 ap=[[0, 64]] + row.ap[1:])
                    nc.gpsimd.dma_start(out=rbs[i][:], in_=bc)

            return [
                lambda: nc.vector.tensor_copy(out=denf[:], in_=denr[:]),
                lambda: nc.vector.reciprocal_approx_fast(
                    out=denf2[:], in_=denf[:]),
                lambda: nc.vector.tensor_copy(out=den16[:], in_=denf2[:]),
                bcasts,
            ]

        def finalize_ops(y_blk, rbs, y16):
            """Micro-op closures: divide each head by its denominator and
            tree-sum into y16.  All f16 (DVE 2x mode)."""
            accs = [fin.tile([64, NMM], f16, tag=f"acc{h}", name=f"acc{h}")
                    for h in range(4)]
            ops = []
            for h in range(4):
                ops.append(lambda h=h: nc.vector.tensor_mul(
                    out=accs[h][:], in0=y_blk[0:64, h, :], in1=rbs[h][:]))
            for h in range(4, 8):
                def mul_add(h=h):
                    t = fin.tile([64, NMM], f16, tag="tmp")
                    nc.vector.tensor_mul(
                        out=t[:], in0=y_blk[0:64, h, :], in1=rbs[h][:])
                    nc.vector.tensor_add(
                        out=accs[h - 4][:], in0=accs[h - 4][:], in1=t[:])
                ops.append(mul_add)
            ops.append(lambda: nc.vector.tensor_add(
                out=accs[0][:], in0=accs[0][:], in1=accs[1][:]))
            ops.append(lambda: nc.vector.tensor_add(
                out=accs[2][:], in0=accs[2][:], in1=accs[3][:]))
            ops.append(lambda: nc.vector.tensor_add(
                out=y16[:], in0=accs[0][:], in1=accs[2][:]))
            return ops

        pend_fin = None     # (iq, y_blk, rbs) awaiting divide+head-sum
        work_q = []         # deferred micro-ops, drip-fed between chunks
        eb_acc = 0.0        # Bresenham accumulator for evac engine choice
        last_st = {}        # last-iq per-pair finalize state

        def last_pair_tail(iq, p, y_blk):
            """Last query block: per-pair den recip + divide + accumulate so
            almost nothing remains after the final PV (short drain tail)."""
            hA, hB = 2 * p, 2 * p + 1
            denr2 = dpool.tile([2, NMM], f16, tag="denr2")
            nc.sync.dma_start(out=denr2[:], in_=dg_d[iq, hA:hA + 2])
            denf2a = dpool.tile([2, NMM], f32, tag="denf2a")
            nc.vector.tensor_copy(out=denf2a[:], in_=denr2[:])
            denf2b = dpool.tile([2, NMM], f32, tag="denf2b")
            nc.vector.reciprocal_approx_fast(out=denf2b[:], in_=denf2a[:])
            den16p = dpool.tile([2, NMM], f16, tag="den16p")
            nc.vector.tensor_copy(out=den16p[:], in_=denf2b[:])
            nc.sync.dma_start(out=den_d[iq, hA:hA + 2], in_=den16p[:])
            rbs2 = []
            for h in (hA, hB):
                rb = fin.tile([64, NMM], f16, tag=f"rbL{h}", name=f"rbL{h}")
                row = den_d[iq, h:h + 1, :]
                bc = bass.AP(tensor=row.tensor, offset=row.offset,
                             ap=[[0, 64]] + row.ap[1:])
                nc.sync.dma_start(out=rb[:], in_=bc)
                rbs2.append(rb)
            if p == 0:
                acc = fin.tile([64, NMM], f16, tag="lacc", name="lacc")
                nc.vector.tensor_mul(
                    out=acc[:], in0=y_blk[0:64, hA, :], in1=rbs2[0][:])
                last_st["acc"] = acc
            else:
                acc = last_st["acc"]
                t = fin.tile([64, NMM], f16, tag="ltmp")
                nc.vector.tensor_mul(
                    out=t[:], in0=y_blk[0:64, hA, :], in1=rbs2[0][:])
                nc.vector.tensor_add(out=acc[:], in0=acc[:], in1=t[:])
            t = fin.tile([64, NMM], f16, tag="ltmp")
            nc.vector.tensor_mul(
                out=t[:], in0=y_blk[0:64, hB, :], in1=rbs2[1][:])
            if p < PAIRS - 1:
                nc.vector.tensor_add(out=acc[:], in0=acc[:], in1=t[:])
            else:
                y16 = blk.tile([64, NMM], f16, tag="y16")
                nc.vector.tensor_add(out=y16[:], in0=acc[:], in1=t[:])
                last_st["y16"] = y16

        for iq in range(IQ):
            i0 = iq * NMM
            y_blk = blk.tile([72, 8, NMM], f16, tag="yblk")
            for p in range(PAIRS):
                if iq == 0:
                    if p == 0:
                        # all Q projections first: they only need xq, so the
                        # PE works while xk is still streaming in
                        for pp in range(PAIRS):
                            proj(pp, True)
                    proj(p, False)
                if p == 0 and pend_fin is not None:
                    # plan the previous block's divide+head-sum+outproj as
                    # micro-ops interleaved into this block's chunk stream
                    fiq, fy_blk, frbs = pend_fin
                    y16 = blk.tile([64, NMM], f16, tag="y16")
                    work_q.extend(finalize_ops(fy_blk, frbs, y16))
                    work_q.extend(outproj_ops(fiq, y16))
                    pend_fin = None
                hA, hB = 2 * p, 2 * p + 1
                yA = ps_y.tile([MV, NMM], f32, tag="yab")
                yB = ps_y.tile([MV, NMM], f32, tag="yab")

                def pv(jp, e2, yA=yA, yB=yB, p=p):
                    # DoubleRow: two key chunks contracted per pass
                    nc.tensor.matmul(
                        out=yA[:],
                        lhsT=vv_sb[p][:, jp, 0, :, :],
                        rhs=e2[:, :, :NMM],
                        start=(jp == 0), stop=(jp == JP - 1),
                        skip_group_check=True,
                        perf_mode=mybir.MatmulPerfMode.DoubleRow,
                    )
                    nc.tensor.matmul(
                        out=yB[:],
                        lhsT=vv_sb[p][:, jp, 1, :, :],
                        rhs=e2[:, :, NMM:],
                        start=(jp == 0), stop=(jp == JP - 1),
                        skip_group_check=True,
                        perf_mode=mybir.MatmulPerfMode.DoubleRow,
                    )

                pend_pv = []
                grace = WQ_GRACE if p == 0 else 0
                for jp in range(JP):
                    e2 = epool.tile([128, 2, 1024], f8, tag="e")
                    for sub in range(2):
                        jc = 2 * jp + sub
                        j0 = jc * 128
                        sAB = ps_mm.tile([128, 1024], f32, tag="mm")
                        nc.tensor.matmul(
                            out=sAB[:, :NMM],
                            lhsT=kT[p][0:64, j0:j0 + 128],
                            rhs=qT[p][0:64, i0:i0 + NMM],
                            start=True, stop=True,
                            tile_position=(0, 0),
                        )
                        nc.tensor.matmul(
                            out=sAB[:, NMM:],
                            lhsT=kT[p][64:128, j0:j0 + 128],
                            rhs=qT[p][64:128, i0:i0 + NMM],
                            start=True, stop=True,
                            tile_position=(64, 0),
                        )
                        # strict per-jp engine alternation (sub0->ACT,
                        # sub1->DVE) so both engines always run one evac per
                        # chunk-pair in parallel; periodically double-up on
                        # ScalarE to hold the 45% DVE ratio (lower while
                        # deferred micro-ops occupy the DVE)
                        eb_acc += 1
                        dub = 3 if (work_q and jp >= grace) else 10
                        use_dve = (sub == 1) and (eb_acc % dub != 0)
                        if use_dve:
                            # Schraudolph exp: one DVE tensor_scalar, f32
                            # PSUM in, int8 out = the fp8e4m3 exp bits
                            nc.vector.tensor_scalar(
                                out=e2[:, sub, :].bitcast(i8),
                                in0=sAB[:],
                                scalar1=SCH_A,
                                scalar2=SCH_B,
                                op0=mybir.AluOpType.mult,
                                op1=mybir.AluOpType.add,
                            )
                        else:
                            nc.scalar.activation(
                                out=e2[:, sub, :], in_=sAB[:],
                                func=mybir.ActivationFunctionType.Exp,
                            )
                        if work_q and jp >= grace and sub == 0:
                            work_q.pop(0)()
                    # defer PV so the in-order PE queue never waits on the
                    # evac latency of the chunk pair it just produced
                    pend_pv.append((jp, e2))
                    if len(pend_pv) > PV_LAG:
                        pv(*pend_pv.pop(0))
                for args in pend_pv:
                    pv(*args)
                # stage numerators + den rows to SBUF f16 (alternate engines)
                nc.scalar.activation(
                    out=y_blk[:, hA, :], in_=yA[0:72, :],
                    func=mybir.ActivationFunctionType.Copy)
                nc.vector.tensor_copy(out=y_blk[:, hB, :], in_=yB[0:72, :])
                # per-pair den gather: spread the iq-end DMA chain
                geng = nc.sync if iq == IQ - 1 else nc.gpsimd
                geng.dma_start(
                    out=dg_d[iq, hA], in_=y_blk[64 + hA:65 + hA, hA, :])
                geng.dma_start(
                    out=dg_d[iq, hB], in_=y_blk[64 + hB:65 + hB, hB, :])
                if iq == IQ - 1:
                    last_pair_tail(iq, p, y_blk)
                elif p == 2:
                    rbs_cur = []
                    work_q.extend(den_chain_ops(iq, 0, 6, rbs_cur))
                elif p == 3:
                    work_q.extend(den_chain_ops(iq, 6, 8, rbs_cur))
            if iq == 0:
                xctx.close()  # release x staging after last projection

            if iq < IQ - 1:
                pend_fin = (iq, y_blk, rbs_cur)
        # drain: leftover micro-ops, then the last block's output projection
        for op in work_q:
            op()
        for op in outproj_ops(IQ - 1, last_st["y16"]):
            op()

    nc.compile()
    return nc


def _prep(queries, keys, values, Wq, bq, Wk, bk, Wv, bv, Wo, bo):
    """Host-side sharding/layout prep. Returns (in_maps, bo_p)."""
    import ml_dtypes
    F8 = ml_dtypes.float8_e4m3

    queries = np.asarray(queries, np.float32)
    keys = np.asarray(keys, np.float32)
    values = np.asarray(values, np.float32)
    Wq = np.asarray(Wq, np.float32)
    bq = np.asarray(bq, np.float32)
    Wk = np.asarray(Wk, np.float32)
    Wv = np.asarray(Wv, np.float32)
    bv = np.asarray(bv, np.float32)
    Wo = np.asarray(Wo, np.float32)
    bo = np.asarray(bo, np.float32)

    scale = 1.0 / np.sqrt(np.float32(D_K))
    Wq_s = Wq * scale
    bq_s = bq * scale
    Wov = Wo @ Wv                       # [1024, 64]
    bo_p = bo + N_HEADS * (Wo @ bv)     # [1024]
    wov_h = np.ascontiguousarray(Wov.T.astype(np.float16))  # [64, 1024]

    in_maps = []
    for c in range(8):
        b = c // 2
        hg = c % 2
        hsl = slice(hg * 512, (hg + 1) * 512)
        # x layout [128, FC, S] fp8 (partition-major for DoubleRow rhs)
        xq = np.ascontiguousarray(
            queries[b].T.astype(F8).reshape(FC, 128, S).transpose(1, 0, 2))
        xk = np.ascontiguousarray(
            keys[b].T.astype(F8).reshape(FC, 128, S).transpose(1, 0, 2))
        wq = np.ascontiguousarray(
            Wq_s[hsl].T.astype(F8).reshape(FC, 128, 512).transpose(1, 0, 2))
        wk = np.ascontiguousarray(
            Wk[hsl].T.astype(F8).reshape(FC, 128, 512).transpose(1, 0, 2))
        bq_c = np.ascontiguousarray(bq_s[hsl].reshape(PAIRS, 128).T)
        # vv[key128, jc, head8, MV] -> [PAIRS, 128, JP, hh, sub, MV] fp8
        vv = np.zeros((128, JC, 8, MV), np.float32)
        vv[:, :, :, :64] = (
            values[b][:, hsl].reshape(JC, 128, 8, 64).transpose(1, 0, 2, 3))
        for h in range(8):
            vv[:, :, h, 64 + h] = 1.0
        vv = vv.reshape(128, JP, 2, PAIRS, 2, MV).transpose(3, 0, 1, 4, 2, 5)
        in_maps.append({
            "xq": xq, "xk": xk, "wq": wq, "wk": wk,
            "bq": bq_c, "vv": np.ascontiguousarray(vv.astype(F8)),
            "wov": wov_h,
        })
    return in_maps, bo_p


def _build_in_maps(inputs):
    return _prep(**inputs)[0]


def _gather(results, bo_p):
    out = np.empty((B, S, N_OUT), np.float32)
    for b in range(B):
        oT = (results[2 * b]["outT"].astype(np.float32)
              + results[2 * b + 1]["outT"].astype(np.float32))
        out[b] = oT.reshape(N_OUT, S).T + bo_p
    return out


def kernel(queries, keys, values, Wq, bq, Wk, bk, Wv, bv, Wo, bo):
    from concourse.bass_utils import run_bass_kernel_spmd

    in_maps, bo_p = _prep(queries, keys, values, Wq, bq, Wk, bk, Wv, bv, Wo, bo)
    if "nc" not in _CACHE:
        _CACHE["nc"] = _build_nc()
    res = run_bass_kernel_spmd(_CACHE["nc"], in_maps, core_ids=list(range(8)))
    return _gather(res.results, bo_p)
